# revision 11
# baseline (speedup 1.0000x reference)
"""Distributed k-NN retrieval kernel for Trainium2 (8 NeuronCores, SPMD).

Math (per the problem): w_i = 1 / (||q - k_i||^2 + delta) over 1M keys;
top-50 w; out = sum_j values[idx_j] * (w_j / sum_i w_i), shape [1, 64].

v2 strategy (fp8 DoubleRow, PE-only scoring). Shard keys row-wise across
8 cores (125000 rows each, padded to 126976 = 4 * 31744). Rows are
scored with
    nd = 2 q . k - ||k||^2   (= -dist + ||q||^2; same ordering as w)
where ||k||^2 is precomputed on the host (q-independent, like the layout
transpose) in bf16, and 2q.k runs on the tensor engine in fp8 e4m3
DoubleRow mode: 256-deep virtual contraction = 4 channels x 64 row
groups, 0.5 cycles per output row. Per 31744-row slice, 16 DoubleRow
matmuls (4 channels each) accumulate 2q.k into a dense [64, 496] PSUM
tile and one bf16 matmul (-I stationary) accumulates -||k||^2 for the
whole slice. This removes v1's ACT/DVE elementwise stage entirely and
cuts PE time 4x (15.9k cycles/core); the kernel becomes DMA-bound at
~8.4MB/core (fp8 keys 8.1MB + bf16 norms 0.25MB) ~= 26us at the
332GB/s roofline. Each slice's 2MB arrives as 8 partition-sliced DMAs
so all 16 DMA engines stream continuously.

Two slices' psum tiles are evacuated (ACT copy) into one [128, 496]
nd_sb bank whose partition p holds a contiguous 496-row bin; one DVE
max8 / max_index pair per bank extracts the top-8 per bin (2048
candidates/core) -- fp8 quantization perturbs nd by sigma~0.9 while
this candidate set covers the exact ranking to depth >=2750 (validated
over seeds; need 50). Final weights are recomputed exactly in fp32 on
the host. The normalization sum S uses the on-device quantized w:
DVE tensor_scalar recovers dist+delta = bias - nd (bias = delta +
||q||^2 as a [128,1] input so the program is q-independent and compiled
once), reciprocal_approx_fast + reduce accumulate it (S rel err ~2e-4,
tolerance 2e-2).

Device-side layout (per core):
  row r in [0, 126976), r = 31744*s + 496*g + f
    s in [0,4)   : slice (one [64,496] psum tile each; bank b = s//2)
    g in [0,64)  : row group (psum partition)
    f in [0,496) : psum free column
  channel c = 4*h + 2*d + i (h in [0,16) matmul pass, d = partition
  bit, i = DoubleRow k-tile). Moving fp8 k8[2g+d, ((2s+i)*16+h)*496+f];
  stationary selq_h[2g+d, i, g'] = fp8(2q[4h+2d+i]) iff g'==g.
"""

import sys

import numpy as np

for _p in ("/opt/trn_rl_repo", "/opt/pypackages"):
    if _p not in sys.path:
        sys.path.insert(0, _p)

DELTA = 0.001
QUERY_WIDTH = 50
N_TOTAL = 1_000_000
D = 64
NCORES = 8
SHARD = N_TOTAL // NCORES  # 125000
FREE = 496                 # psum free columns
NG = 64                    # row groups per slice
NS = 4                     # slices per core
NH = 16                    # fp8 matmul passes per slice (4 channels each)
SROWS = NG * FREE          # 31744 rows per slice
RPAD = NS * SROWS          # 126976 padded rows per core
NB = 2                     # candidate banks (2 slices each)
PAD_N2 = 1.0e9             # ||k||^2 for padding rows -> nd ~ -1e9


def _build_nc():
    import concourse.bacc as bacc
    import concourse.mybir as mybir
    import concourse.tile as tile

    nc = bacc.Bacc(None, target_bir_lowering=False)

    # Moving keys, fp8 e4m3: [2g+d, ((2s+i)*16+h)*496+f]
    k8 = nc.dram_tensor("k8", [128, 2 * NS * NH * FREE], mybir.dt.float8e4,
                        kind="ExternalInput")
    # Row norms, bf16: [g, 496*s+f]
    nrm = nc.dram_tensor("nrm", [NG, NS * FREE], mybir.dt.bfloat16,
                         kind="ExternalInput")
    # Stationary 2q selectors: [2g+d, (h, i, g')] packed per pass h
    selq = nc.dram_tensor("selq", [128, NH * 128], mybir.dt.float8e4,
                          kind="ExternalInput")
    # Norm selector -I[64]
    seln = nc.dram_tensor("seln", [NG, NG], mybir.dt.bfloat16,
                          kind="ExternalInput")
    # bias = DELTA + ||q||^2, broadcast [128, 1]
    biasq = nc.dram_tensor("biasq", [128, 1], mybir.dt.float32,
                           kind="ExternalInput")
    out_cand = nc.dram_tensor("out_cand", [128, NB * 8], mybir.dt.uint32,
                              kind="ExternalOutput")
    out_wacc = nc.dram_tensor("out_wacc", [128, NB], mybir.dt.float32,
                              kind="ExternalOutput")

    DR = mybir.MatmulPerfMode.DoubleRow
    SLW = 2 * NH * FREE  # 15872 k8 columns per slice

    with tile.TileContext(nc) as tc:
        with (
            tc.tile_pool(name="consts", bufs=1) as consts,
            tc.tile_pool(name="kpool", bufs=4) as kpool,
            tc.tile_pool(name="wpool", bufs=1) as wpool,
            tc.tile_pool(name="bpool", bufs=2) as bpool,
            tc.tile_pool(name="psum", bufs=5, space="PSUM") as psum,
        ):
            # Issue the full k8 stream first: 16 DMAs ([128, 3968] each,
            # ~3.9KB contiguous per partition), one per DMA engine, so all
            # engines run concurrently from t~0. Consts dispatch after.
            kts = []
            CW = SLW // 4  # 3968
            for s in range(NS):
                kt = kpool.tile([128, SLW], mybir.dt.float8e4, tag="ktile",
                                name=f"kt{s}")
                for m in range(4):
                    nc.sync.dma_start(
                        out=kt[:, CW * m: CW * (m + 1)],
                        in_=k8[:, SLW * s + CW * m: SLW * s + CW * (m + 1)],
                    )
                kts.append(kt)

            selq_sb = consts.tile([128, NH * 128], mybir.dt.float8e4,
                                  tag="sq")
            seln_sb = consts.tile([NG, NG], mybir.dt.bfloat16, tag="sn")
            biasq_sb = consts.tile([128, 1], mybir.dt.float32, tag="bq")
            nrm_sb = consts.tile([NG, NS * FREE], mybir.dt.bfloat16,
                                 tag="nrm")
            nc.sync.dma_start(out=selq_sb[:], in_=selq[:])
            nc.sync.dma_start(out=seln_sb[:], in_=seln[:])
            nc.sync.dma_start(out=biasq_sb[:], in_=biasq[:])
            nc.sync.dma_start(out=nrm_sb[:], in_=nrm[:])

            def lq(h):
                return selq_sb[:, 128 * h: 128 * (h + 1)].rearrange(
                    "p (i g) -> p i g", i=2
                )

            cand_sb = wpool.tile([128, NB * 8], mybir.dt.uint32, tag="cand")
            wacc_sb = wpool.tile([128, NB], mybir.dt.float32, tag="wacc")

            # PE warmup: junk DoubleRow matmuls during the first k8 DMA so
            # the clock-gate ramps toward full rate before the real stream.
            wrm = wpool.tile([128, 2 * FREE], mybir.dt.float8e4, tag="wrm")
            wps = psum.tile([NG, FREE], mybir.dt.float32, tag="wps",
                            name="wps", bufs=1)
            nc.vector.memset(wrm[:], 0.0)
            wrm3 = wrm.rearrange("p (i u) -> p i u", i=2)
            for _w in range(10):
                nc.tensor.matmul(wps[:], lq(0), wrm3, start=True, stop=True,
                                 perf_mode=DR, tile_position=(0, 0))

            nd_sb = [None, None]
            for s in range(NS):
                kt3 = kts[s].rearrange("p (i hf) -> p i hf", i=2)

                pt = psum.tile([NG, FREE], mybir.dt.float32, tag="ps",
                               name=f"ps{s}")
                for h in range(NH):
                    rhs = kt3[:, :, FREE * h: FREE * (h + 1)]
                    nc.tensor.matmul(
                        pt[:], lq(h), rhs, start=(h == 0), stop=False,
                        perf_mode=DR, tile_position=(0, 0),
                        skip_group_check=True,
                    )
                # -||k||^2 for the whole slice in one bf16 matmul.
                nc.tensor.matmul(
                    pt[:], seln_sb[:], nrm_sb[:, FREE * s: FREE * (s + 1)],
                    start=False, stop=True, tile_position=(0, 0),
                    skip_group_check=True,
                )

                # Trickle of dummy matmuls between slices keeps the PE
                # clock-gate ramped while the next slice streams in.
                if s < NS - 1:
                    for _w in range(24):
                        nc.tensor.matmul(wps[:], lq(0), wrm3, start=True,
                                         stop=True, perf_mode=DR,
                                         tile_position=(0, 0))

                b, half = divmod(s, 2)
                if half == 0:
                    nd_sb[b] = bpool.tile([128, FREE], mybir.dt.float32,
                                          tag="ndsb", name=f"nd{b}")
                nc.scalar.copy(nd_sb[b][64 * half: 64 * (half + 1), :],
                               pt[:])

                if half == 1:
                    nd = nd_sb[b]
                    mx = bpool.tile([128, 8], mybir.dt.float32, tag="mx")
                    nc.vector.max(mx[:], nd[:])
                    nc.vector.max_index(cand_sb[:, 8 * b: 8 * (b + 1)],
                                        mx[:], nd[:])
                    # Partial sum of w = 1/(bias - nd), ~18-bit exact.
                    dpd = bpool.tile([128, FREE], mybir.dt.float32,
                                     tag="dpd")
                    nc.vector.tensor_scalar(
                        out=dpd[:], in0=nd[:], scalar1=-1.0,
                        scalar2=biasq_sb[:, 0:1],
                        op0=mybir.AluOpType.mult, op1=mybir.AluOpType.add,
                    )
                    wv = bpool.tile([128, FREE], mybir.dt.float32, tag="wv")
                    nc.vector.reciprocal_approx_fast(wv[:], dpd[:])
                    nc.vector.tensor_reduce(
                        out=wacc_sb[:, b: b + 1], in_=wv[:],
                        axis=mybir.AxisListType.X, op=mybir.AluOpType.add,
                    )

            nc.sync.dma_start(out=out_cand[:], in_=cand_sb[:])
            nc.sync.dma_start(out=out_wacc[:], in_=wacc_sb[:])

    nc.compile()
    return nc


def _host_inputs(q: np.ndarray, keys: np.ndarray):
    """Build per-core DRAM inputs (fp8 keys layout + bf16 row norms)."""
    import ml_dtypes

    f8 = ml_dtypes.float8_e4m3
    bf = ml_dtypes.bfloat16

    # selq[2g+d, 128h + (8i? no: (i,g'))]: col = 128*h + 64*i + g'
    q8 = (2.0 * q.astype(np.float32)).astype(f8)
    selq = np.zeros((128, NH * 128), f8)
    g = np.arange(NG)
    for h in range(NH):
        for d in range(2):
            for i in range(2):
                selq[2 * g + d, 128 * h + 64 * i + g] = q8[4 * h + 2 * d + i]
    seln = np.zeros((NG, NG), bf)
    seln[np.arange(NG), np.arange(NG)] = bf(-1.0)
    bias_const = np.float32(DELTA) + (q.astype(np.float32) ** 2).sum(
        dtype=np.float32
    )
    biasq = np.full((128, 1), bias_const, np.float32)

    in_maps = []
    for c in range(NCORES):
        shard = keys[c * SHARD: (c + 1) * SHARD]
        pad = np.zeros((RPAD, D), np.float32)
        pad[:SHARD] = shard
        n2 = (pad * pad).sum(axis=1, dtype=np.float32)
        n2[SHARD:] = PAD_N2
        # rows (s, g, f) x chans (h, d, i) -> [(g,d), (s,i,h,f)]
        k8 = np.ascontiguousarray(
            pad.astype(f8)
            .reshape(NS, NG, FREE, NH, 2, 2)        # s g f h d i
            .transpose(1, 4, 0, 5, 3, 2)            # g d s i h f
            .reshape(128, 2 * NS * NH * FREE)
        )
        nrm = np.ascontiguousarray(
            n2.astype(bf)
            .reshape(NS, NG, FREE)
            .transpose(1, 0, 2)
            .reshape(NG, NS * FREE)
        )
        in_maps.append({
            "k8": k8, "nrm": nrm, "selq": selq, "seln": seln, "biasq": biasq,
        })
    return in_maps


def decode_rows(cand: np.ndarray, b: int) -> np.ndarray:
    """Decode bank b's candidates to shard rows: partition P, col v ->
    r = 31744*(2b + P//64) + 496*(P%64) + v."""
    v = cand[:, 8 * b: 8 * (b + 1)].astype(np.int64)
    p = np.arange(128)[:, None]
    rows = SROWS * (2 * b + p // 64) + FREE * (p % 64) + v
    rows[(v < 0) | (v >= FREE)] = RPAD
    return rows.reshape(-1)


def _merge(results, q: np.ndarray, keys: np.ndarray, values: np.ndarray):
    """Host-side gather/unshard: exact top-50 over the candidate superset."""
    S = np.float32(
        sum(np.asarray(r["out_wacc"], np.float64).sum() for r in results)
    )
    g_list = []
    for c, r in enumerate(results):
        cand = np.asarray(r["out_cand"])  # [128, NB*8] uint32
        for b in range(NB):
            rows = decode_rows(cand, b)
            rows = rows[rows < SHARD]
            g_list.append(c * SHARD + rows)
    g = np.unique(np.concatenate(g_list))
    # exact fp32 recompute of candidate weights
    diff = q[None, :] - keys[g]
    d = (diff * diff).sum(axis=1, dtype=np.float32)
    w = np.float32(1.0) / (d + np.float32(DELTA))
    order = np.lexsort((g, -w))  # descending w, ties by lower global index
    sel = order[:QUERY_WIDTH]
    weights = (w[sel] / S).astype(np.float32)[:, None]
    out = (values[g[sel]] * weights).sum(axis=0, keepdims=True,
                                         dtype=np.float32)
    return out.astype(np.float32)


_NC_CACHE: list = []


def _get_nc():
    if not _NC_CACHE:
        _NC_CACHE.append(_build_nc())
    return _NC_CACHE[0]


def kernel(key, keys, values):
    from concourse.bass_utils import run_bass_kernel_spmd

    q = np.ascontiguousarray(np.asarray(key, np.float32))
    K = np.ascontiguousarray(np.asarray(keys, np.float32))
    V = np.ascontiguousarray(np.asarray(values, np.float32))
    assert q.shape == (D,) and K.shape == (N_TOTAL, D) and V.shape == (N_TOTAL, D)

    nc = _get_nc()
    in_maps = _host_inputs(q, K)
    res = run_bass_kernel_spmd(nc, in_maps, list(range(NCORES))).results
    return _merge(res, q, K, V)


# revision 14
# speedup vs baseline: 1.5528x; 1.5528x over previous
"""Distributed k-NN retrieval kernel for Trainium2 (8 NeuronCores, SPMD).

Math (per the problem): w_i = 1 / (||q - k_i||^2 + delta) over 1M keys;
top-50 w; out = sum_j values[idx_j] * (w_j / sum_i w_i), shape [1, 64].

v2 strategy (fp8 DoubleRow, PE-only scoring). Shard keys row-wise across
8 cores (125000 rows each, padded to 126976 = 4 * 31744). Rows are
scored with
    nd = 2 q . k - ||k||^2   (= -dist + ||q||^2; same ordering as w)
where ||k||^2 is precomputed on the host (q-independent, like the layout
transpose) in bf16, and 2q.k runs on the tensor engine in fp8 e4m3
DoubleRow mode: 256-deep virtual contraction = 4 channels x 64 row
groups, 0.5 cycles per output row. Per 31744-row slice, 16 DoubleRow
matmuls (4 channels each) accumulate 2q.k into a dense [64, 496] PSUM
tile and one bf16 matmul (-I stationary) accumulates -||k||^2 for the
whole slice. This removes v1's ACT/DVE elementwise stage entirely and
cuts PE time 4x (15.9k cycles/core); the kernel becomes DMA-bound at
~8.4MB/core (fp8 keys 8.1MB + bf16 norms 0.25MB) ~= 26us at the
332GB/s roofline. Each slice's 2MB arrives as 8 partition-sliced DMAs
so all 16 DMA engines stream continuously.

Two slices' psum tiles are evacuated (ACT copy) into one [128, 496]
nd_sb bank whose partition p holds a contiguous 496-row bin; one DVE
max8 / max_index pair per bank extracts the top-8 per bin (2048
candidates/core) -- fp8 quantization perturbs nd by sigma~0.9 while
this candidate set covers the exact ranking to depth >=2750 (validated
over seeds; need 50). Final weights are recomputed exactly in fp32 on
the host. The normalization sum S uses the on-device quantized w:
DVE tensor_scalar recovers dist+delta = bias - nd (bias = delta +
||q||^2 as a [128,1] input so the program is q-independent and compiled
once), reciprocal_approx_fast + reduce accumulate it (S rel err ~2e-4,
tolerance 2e-2).

Device-side layout (per core):
  row r in [0, 126976), r = 31744*s + 496*g + f
    s in [0,4)   : slice (one [64,496] psum tile each; bank b = s//2)
    g in [0,64)  : row group (psum partition)
    f in [0,496) : psum free column
  channel c = 4*h + 2*d + i (h in [0,16) matmul pass, d = partition
  bit, i = DoubleRow k-tile). Moving fp8 k8[2g+d, ((2s+i)*16+h)*496+f];
  stationary selq_h[2g+d, i, g'] = fp8(2q[4h+2d+i]) iff g'==g.
"""

import sys

import numpy as np

for _p in ("/opt/trn_rl_repo", "/opt/pypackages"):
    if _p not in sys.path:
        sys.path.insert(0, _p)

DELTA = 0.001
QUERY_WIDTH = 50
N_TOTAL = 1_000_000
D = 64
NCORES = 8
SHARD = N_TOTAL // NCORES  # 125000
FREE = 496                 # psum free columns
NG = 64                    # row groups per slice
NS = 4                     # slices per core
NH = 16                    # fp8 matmul passes per slice (4 channels each)
SROWS = NG * FREE          # 31744 rows per slice
RPAD = NS * SROWS          # 126976 padded rows per core
NB = 2                     # candidate banks (2 slices each)
PAD_N2 = 1.0e9             # ||k||^2 for padding rows -> nd ~ -1e9


def _build_nc():
    import concourse.bacc as bacc
    import concourse.mybir as mybir
    import concourse.tile as tile

    nc = bacc.Bacc(None, target_bir_lowering=False)

    # Moving keys, fp8 e4m3: [2g+d, ((2s+i)*16+h)*496+f]
    k8 = nc.dram_tensor("k8", [128, 2 * NS * NH * FREE], mybir.dt.float8e4,
                        kind="ExternalInput")
    # Row norms, bf16: [g, 496*s+f]
    nrm = nc.dram_tensor("nrm", [NG, NS * FREE], mybir.dt.bfloat16,
                         kind="ExternalInput")
    # Stationary 2q selectors: [2g+d, (h, i, g')] packed per pass h
    selq = nc.dram_tensor("selq", [128, NH * 128], mybir.dt.float8e4,
                          kind="ExternalInput")
    # Norm selector -I[64]
    seln = nc.dram_tensor("seln", [NG, NG], mybir.dt.bfloat16,
                          kind="ExternalInput")
    # bias = DELTA + ||q||^2, broadcast [128, 1]
    biasq = nc.dram_tensor("biasq", [128, 1], mybir.dt.float32,
                           kind="ExternalInput")
    out_cand = nc.dram_tensor("out_cand", [128, NB * 8], mybir.dt.uint32,
                              kind="ExternalOutput")
    out_wacc = nc.dram_tensor("out_wacc", [128, NB], mybir.dt.float32,
                              kind="ExternalOutput")

    DR = mybir.MatmulPerfMode.DoubleRow
    SLW = 2 * NH * FREE  # 15872 k8 columns per slice

    with tile.TileContext(nc) as tc:
        with (
            tc.tile_pool(name="consts", bufs=1) as consts,
            tc.tile_pool(name="kpool", bufs=4) as kpool,
            tc.tile_pool(name="wpool", bufs=1) as wpool,
            tc.tile_pool(name="bpool", bufs=2) as bpool,
            tc.tile_pool(name="psum", bufs=5, space="PSUM") as psum,
        ):
            selq_sb = consts.tile([128, NH * 128], mybir.dt.float8e4,
                                  tag="sq")
            seln_sb = consts.tile([NG, NG], mybir.dt.bfloat16, tag="sn")
            biasq_sb = consts.tile([128, 1], mybir.dt.float32, tag="bq")
            nrm_sb = consts.tile([NG, NS * FREE], mybir.dt.bfloat16,
                                 tag="nrm")
            nc.sync.dma_start(out=selq_sb[:], in_=selq[:])
            nc.sync.dma_start(out=seln_sb[:], in_=seln[:])
            nc.sync.dma_start(out=biasq_sb[:], in_=biasq[:])
            nc.sync.dma_start(out=nrm_sb[:], in_=nrm[:])

            def lq(h):
                return selq_sb[:, 128 * h: 128 * (h + 1)].rearrange(
                    "p (i g) -> p i g", i=2
                )

            cand_sb = wpool.tile([128, NB * 8], mybir.dt.uint32, tag="cand")
            wacc_sb = wpool.tile([128, NB], mybir.dt.float32, tag="wacc")

            # PE warmup: junk DoubleRow matmuls during the first k8 DMA so
            # the clock-gate ramps toward full rate before the real stream.
            wrm = wpool.tile([128, 2 * FREE], mybir.dt.float8e4, tag="wrm")
            wps = psum.tile([NG, FREE], mybir.dt.float32, tag="wps",
                            name="wps", bufs=1)
            nc.vector.memset(wrm[:], 0.0)
            wrm3 = wrm.rearrange("p (i u) -> p i u", i=2)
            for _w in range(10):
                nc.tensor.matmul(wps[:], lq(0), wrm3, start=True, stop=True,
                                 perf_mode=DR, tile_position=(0, 0))

            nd_sb = [None, None]
            for s in range(NS):
                kt = kpool.tile([128, SLW], mybir.dt.float8e4, tag="ktile")
                # 8 column-sliced DMAs ([128, 1984] each, ~2KB contiguous
                # per partition) so the stream spreads across the DMA
                # engines at full per-engine efficiency.
                CW = SLW // 8  # 1984
                for m in range(8):
                    nc.sync.dma_start(
                        out=kt[:, CW * m: CW * (m + 1)],
                        in_=k8[:, SLW * s + CW * m: SLW * s + CW * (m + 1)],
                    )
                kt3 = kt.rearrange("p (i hf) -> p i hf", i=2)

                pt = psum.tile([NG, FREE], mybir.dt.float32, tag="ps",
                               name=f"ps{s}")
                for h in range(NH):
                    rhs = kt3[:, :, FREE * h: FREE * (h + 1)]
                    nc.tensor.matmul(
                        pt[:], lq(h), rhs, start=(h == 0), stop=False,
                        perf_mode=DR, tile_position=(0, 0),
                        skip_group_check=True,
                    )
                # -||k||^2 for the whole slice in one bf16 matmul.
                nc.tensor.matmul(
                    pt[:], seln_sb[:], nrm_sb[:, FREE * s: FREE * (s + 1)],
                    start=False, stop=True, tile_position=(0, 0),
                    skip_group_check=True,
                )

                b, half = divmod(s, 2)
                if half == 0:
                    nd_sb[b] = bpool.tile([128, FREE], mybir.dt.float32,
                                          tag="ndsb", name=f"nd{b}")
                nc.scalar.copy(nd_sb[b][64 * half: 64 * (half + 1), :],
                               pt[:])

                if half == 1:
                    nd = nd_sb[b]
                    mx = bpool.tile([128, 8], mybir.dt.float32, tag="mx")
                    nc.vector.max(mx[:], nd[:])
                    nc.vector.max_index(cand_sb[:, 8 * b: 8 * (b + 1)],
                                        mx[:], nd[:])
                    # Partial sum of w = 1/(bias - nd), ~18-bit exact.
                    dpd = bpool.tile([128, FREE], mybir.dt.float32,
                                     tag="dpd")
                    nc.vector.tensor_scalar(
                        out=dpd[:], in0=nd[:], scalar1=-1.0,
                        scalar2=biasq_sb[:, 0:1],
                        op0=mybir.AluOpType.mult, op1=mybir.AluOpType.add,
                    )
                    wv = bpool.tile([128, FREE], mybir.dt.float32, tag="wv")
                    nc.vector.reciprocal_approx_fast(wv[:], dpd[:])
                    nc.vector.tensor_reduce(
                        out=wacc_sb[:, b: b + 1], in_=wv[:],
                        axis=mybir.AxisListType.X, op=mybir.AluOpType.add,
                    )

            nc.sync.dma_start(out=out_cand[:], in_=cand_sb[:])
            nc.sync.dma_start(out=out_wacc[:], in_=wacc_sb[:])

    nc.compile()
    return nc


def _host_inputs(q: np.ndarray, keys: np.ndarray):
    """Build per-core DRAM inputs (fp8 keys layout + bf16 row norms)."""
    import ml_dtypes

    f8 = ml_dtypes.float8_e4m3
    bf = ml_dtypes.bfloat16

    # selq[2g+d, 128h + (8i? no: (i,g'))]: col = 128*h + 64*i + g'
    q8 = (2.0 * q.astype(np.float32)).astype(f8)
    selq = np.zeros((128, NH * 128), f8)
    g = np.arange(NG)
    for h in range(NH):
        for d in range(2):
            for i in range(2):
                selq[2 * g + d, 128 * h + 64 * i + g] = q8[4 * h + 2 * d + i]
    seln = np.zeros((NG, NG), bf)
    seln[np.arange(NG), np.arange(NG)] = bf(-1.0)
    bias_const = np.float32(DELTA) + (q.astype(np.float32) ** 2).sum(
        dtype=np.float32
    )
    biasq = np.full((128, 1), bias_const, np.float32)

    in_maps = []
    for c in range(NCORES):
        shard = keys[c * SHARD: (c + 1) * SHARD]
        pad = np.zeros((RPAD, D), np.float32)
        pad[:SHARD] = shard
        n2 = (pad * pad).sum(axis=1, dtype=np.float32)
        n2[SHARD:] = PAD_N2
        # rows (s, g, f) x chans (h, d, i) -> [(g,d), (s,i,h,f)]
        k8 = np.ascontiguousarray(
            pad.astype(f8)
            .reshape(NS, NG, FREE, NH, 2, 2)        # s g f h d i
            .transpose(1, 4, 0, 5, 3, 2)            # g d s i h f
            .reshape(128, 2 * NS * NH * FREE)
        )
        nrm = np.ascontiguousarray(
            n2.astype(bf)
            .reshape(NS, NG, FREE)
            .transpose(1, 0, 2)
            .reshape(NG, NS * FREE)
        )
        in_maps.append({
            "k8": k8, "nrm": nrm, "selq": selq, "seln": seln, "biasq": biasq,
        })
    return in_maps


def decode_rows(cand: np.ndarray, b: int) -> np.ndarray:
    """Decode bank b's candidates to shard rows: partition P, col v ->
    r = 31744*(2b + P//64) + 496*(P%64) + v."""
    v = cand[:, 8 * b: 8 * (b + 1)].astype(np.int64)
    p = np.arange(128)[:, None]
    rows = SROWS * (2 * b + p // 64) + FREE * (p % 64) + v
    rows[(v < 0) | (v >= FREE)] = RPAD
    return rows.reshape(-1)


def _merge(results, q: np.ndarray, keys: np.ndarray, values: np.ndarray):
    """Host-side gather/unshard: exact top-50 over the candidate superset."""
    S = np.float32(
        sum(np.asarray(r["out_wacc"], np.float64).sum() for r in results)
    )
    g_list = []
    for c, r in enumerate(results):
        cand = np.asarray(r["out_cand"])  # [128, NB*8] uint32
        for b in range(NB):
            rows = decode_rows(cand, b)
            rows = rows[rows < SHARD]
            g_list.append(c * SHARD + rows)
    g = np.unique(np.concatenate(g_list))
    # exact fp32 recompute of candidate weights
    diff = q[None, :] - keys[g]
    d = (diff * diff).sum(axis=1, dtype=np.float32)
    w = np.float32(1.0) / (d + np.float32(DELTA))
    order = np.lexsort((g, -w))  # descending w, ties by lower global index
    sel = order[:QUERY_WIDTH]
    weights = (w[sel] / S).astype(np.float32)[:, None]
    out = (values[g[sel]] * weights).sum(axis=0, keepdims=True,
                                         dtype=np.float32)
    return out.astype(np.float32)


_NC_CACHE: list = []


def _get_nc():
    if not _NC_CACHE:
        _NC_CACHE.append(_build_nc())
    return _NC_CACHE[0]


def kernel(key, keys, values):
    from concourse.bass_utils import run_bass_kernel_spmd

    q = np.ascontiguousarray(np.asarray(key, np.float32))
    K = np.ascontiguousarray(np.asarray(keys, np.float32))
    V = np.ascontiguousarray(np.asarray(values, np.float32))
    assert q.shape == (D,) and K.shape == (N_TOTAL, D) and V.shape == (N_TOTAL, D)

    nc = _get_nc()
    in_maps = _host_inputs(q, K)
    res = run_bass_kernel_spmd(nc, in_maps, list(range(NCORES))).results
    return _merge(res, q, K, V)
